# revision 3
# baseline (speedup 1.0000x reference)
"""Trainium2 Bass kernel for nn_Attention_preprocessor (gnn_message_passing).

Pure data parallel: batch dim B=8192 sharded across the 8 NeuronCores
(1024 batches/core); the small weight matrices are replicated.  The
per-core program is a hand-written Bass/Tile kernel (see bass_kernel.py
when present; the builder is inlined below so this file is self-contained):

  h  = silu(x @ W_in.T + b_in) * lap
  Q/K (token layout, f32 - they feed exp), V (hidden layout, bf16)
  S[i,j] = sum_n Q[n,i] K[n,j]; P = exp(S - rowmax); attn = (P @ V)/Z
  y = silu(attn @ W_out.T + b_out) * lap; out = mean_n y

Layout highlights: batches pair (b, b+B/2) on SBUF partition halves
("2-stack"), diagonal-duplicated weights make every shared matmul dense,
S is computed in (i,j) orientation for free-axis softmax reductions, P is
transposed per pair on the TensorEngine with junk-tolerant (128,128)
transposes, and the attention matmul consumes a zero-structured V-spread
with ones-columns that produce the softmax denominators for free.
"""

from contextlib import ExitStack

import numpy as np

import concourse.bass as bass
import concourse.bacc as bacc
import concourse.mybir as mybir
from concourse.tile import TileContext

F32 = mybir.dt.float32
BF16 = mybir.dt.bfloat16
I32 = mybir.dt.int32
AF = mybir.ActivationFunctionType
ALU = mybir.AluOpType
AX = mybir.AxisListType

NCORES = 8
B, NA, DIN, H, DOUT = 8192, 32, 8, 64, 8
BS = B // NCORES


def build_nc(bs: int, reps: int = 1) -> bass.Bass:
    assert bs % 256 == 0
    HALF = bs // 2
    NST = bs // 128  # supertiles of 128 batches (64 even-half + 64 odd-half)
    THALF = HALF * 32  # tokens per half

    nc = bacc.Bacc("TRN2", target_bir_lowering=False, debug=False)

    # xT: [16, THALF] f32: rows 0:8 = x^T of even-half tokens, 8:16 = odd-half
    x = nc.declare_dram_parameter("xT", [16, THALF], F32, isOutput=False)
    lap = nc.declare_dram_parameter("lap", [2, THALF], F32, isOutput=False)
    w2d = nc.declare_dram_parameter("W2d", [16, 128], F32, isOutput=False)
    bin2d = nc.declare_dram_parameter("bin2d", [128, 1], F32, isOutput=False)
    aqd = nc.declare_dram_parameter("AqDd", [128, 128], F32, isOutput=False)
    akd = nc.declare_dram_parameter("AkDd", [128, 128], F32, isOutput=False)
    avd = nc.declare_dram_parameter("AvDd", [128, 128], F32, isOutput=False)
    wod = nc.declare_dram_parameter("WoDd", [128, 16], BF16, isOutput=False)
    bout2d = nc.declare_dram_parameter("bout2d", [16, 1], F32, isOutput=False)
    # out rows = (half, o), cols = batch within half; host reassembles
    out = nc.declare_dram_parameter("out", [16, HALF], F32, isOutput=True)

    with ExitStack() as ctx:
        tc = ctx.enter_context(TileContext(nc))
        cpool = ctx.enter_context(tc.tile_pool(name="const", bufs=1))
        spool = ctx.enter_context(tc.tile_pool(name="sb", bufs=2))
        ppool = ctx.enter_context(tc.tile_pool(name="ps", bufs=1, space="PSUM"))
        s1pool = ctx.enter_context(tc.tile_pool(name="s1p", bufs=1, space="PSUM"))
        apool = ctx.enter_context(tc.tile_pool(name="ap", bufs=1, space="PSUM"))

        # ---------------- constants (host-prebuilt, one DMA each) --------
        W2 = cpool.tile([16, 128], F32, tag="w2")
        nc.sync.dma_start(out=W2[:], in_=w2d[:])
        AqD = cpool.tile([128, 128], F32, tag="aqd")
        nc.sync.dma_start(out=AqD[:], in_=aqd[:])
        AkD = cpool.tile([128, 128], F32, tag="akd")
        nc.sync.dma_start(out=AkD[:], in_=akd[:])
        AvD = cpool.tile([128, 128], F32, tag="avd")
        nc.sync.dma_start(out=AvD[:], in_=avd[:])
        WoutD = cpool.tile([128, 16], BF16, tag="woutd")
        nc.sync.dma_start(out=WoutD[:], in_=wod[:])
        bin2 = cpool.tile([128, 1], F32, tag="bin2")
        nc.sync.dma_start(out=bin2[:], in_=bin2d[:])
        bout2 = cpool.tile([16, 1], F32, tag="bout2")
        nc.sync.dma_start(out=bout2[:], in_=bout2d[:])

        # identities for PE transposes
        itf = cpool.tile([128, 128], I32, tag="itf")
        itp = cpool.tile([128, 128], I32, tag="itp")
        nc.gpsimd.iota(itf[:], [[1, 128]], channel_multiplier=0)
        nc.gpsimd.iota(itp[:], [[0, 128]], channel_multiplier=1)
        id128 = cpool.tile([128, 128], BF16, tag="id128")
        nc.vector.tensor_tensor(out=id128[:], in0=itf[:], in1=itp[:], op=ALU.is_equal)

        # PE semaphore warm-up: matmuls lower to LDW+MM with tight wait
        # budgets; pre-observe each constant's DMA semaphore.
        dmy = ppool.tile([128, 512], F32, tag="hty")
        nc.tensor.matmul(out=dmy[0:128, 0:1], lhsT=W2[:], rhs=W2[:, 0:1])
        nc.tensor.matmul(out=dmy[0:128, 1:2], lhsT=AqD[:], rhs=AqD[:, 0:1])
        nc.tensor.matmul(out=dmy[0:128, 2:3], lhsT=AkD[:], rhs=AkD[:, 0:1])
        nc.tensor.matmul(out=dmy[0:128, 3:4], lhsT=AvD[:], rhs=AvD[:, 0:1])
        nc.tensor.matmul(out=dmy[0:16, 4:5], lhsT=WoutD[:], rhs=WoutD[:, 0:1])
        nc.tensor.matmul(out=dmy[0:128, 5:6], lhsT=id128[:], rhs=id128[:, 0:1])

        # K-spread ping-pong: zero background persists, K blocks overwritten
        ksp = []
        for i in range(2):
            t = cpool.tile([128, 2048], F32, tag=f"ksp{i}")
            nc.vector.memset(t[:], 0.0)
            ksp.append(t)

        # pair-adjacent P (post-exp) ping-pong; junk quadrants zeroed once
        p1q = []
        for i in range(2):
            t = cpool.tile([128, 2048], BF16, tag=f"p1q{i}")
            nc.vector.memset(t[:], 0.0)
            p1q.append(t)

        # V-spread ping-pong: per-pair slot of 66 cols:
        #   [0:32]=V_be (top), [32]=ones(top), [33:65]=V_bo (bottom), [65]=ones(bottom)
        vsp = []
        for i in range(2):
            t = cpool.tile([128, 66 * 64], BF16, tag=f"vsp{i}")
            nc.vector.memset(t[:], 0.0)
            tv_top = t[0:64, :].rearrange("p (q r) -> p q r", r=66)
            tv_bot = t[64:128, :].rearrange("p (q r) -> p q r", r=66)
            nc.vector.memset(tv_top[:, :, 32:33], 1.0)
            nc.vector.memset(tv_bot[:, :, 65:66], 1.0)
            vsp.append(t)

        yall = cpool.tile([16, bs // 2], F32, tag="yall")

        # ---------------- main loop over supertiles ----------------
        for s_rep in range(NST * reps):
            s = s_rep % NST
            tok0 = 2048 * s  # token offset within each half

            xT2 = spool.tile([16, 2048], F32, tag="xt2")
            nc.gpsimd.dma_start(out=xT2[:], in_=x[:, tok0 : tok0 + 2048])

            lap128 = spool.tile([128, 2048], F32, tag="lap128")
            lsrc = bass.AP(lap[:].tensor, tok0, [[THALF, 2], [0, 64], [1, 2048]])
            nc.gpsimd.dma_start(
                out=lap128[:].rearrange("p (a t) -> p a t", a=1), in_=lsrc
            )

            # ---- layer 1: hT = silu(.) * lap, (128, 2048) f32, 2-stacked
            hT = spool.tile([128, 2048], F32, tag="ht")
            for q in range(4):
                htp = ppool.tile([128, 512], F32, tag="hty")
                nc.tensor.matmul(
                    out=htp[:], lhsT=W2[:], rhs=xT2[:, 512 * q : 512 * (q + 1)]
                )
                nc.scalar.activation(
                    out=hT[:, 512 * q : 512 * (q + 1)],
                    in_=htp[:],
                    func=AF.Silu,
                    bias=bin2[:],
                )
            nc.vector.tensor_tensor(out=hT[:], in0=hT[:], in1=lap128[:], op=ALU.mult)

            anrm = spool.tile([128, 2048], BF16, tag="anrm")
            asil = spool.tile([128, 2048], BF16, tag="asil")
            vs = vsp[s_rep % 2]

            for q2 in range(4):
                # ---- Qt / K token-layout chunks (4 chunks of 128 token-pairs)
                qtp = ppool.tile([128, 512], F32, tag="qt")
                ktp = ppool.tile([128, 512], F32, tag="kt")
                for cc in range(4):
                    chunk = 4 * q2 + cc
                    hchunk = hT[:, 128 * chunk : 128 * (chunk + 1)]
                    nc.tensor.matmul(
                        out=qtp[:, 128 * cc : 128 * (cc + 1)], lhsT=hchunk, rhs=AqD[:]
                    )
                    nc.tensor.matmul(
                        out=ktp[:, 128 * cc : 128 * (cc + 1)], lhsT=hchunk, rhs=AkD[:]
                    )
                qts = spool.tile([128, 512], F32, tag="qts")
                kts = spool.tile([128, 512], F32, tag="kts")
                nc.scalar.activation(out=qts[:], in_=qtp[:], func=AF.Silu)
                nc.scalar.activation(out=kts[:], in_=ktp[:], func=AF.Silu)

                # ---- V in hidden-layout (2-stacked) with fused spread-evict
                vtp = ppool.tile([128, 512], F32, tag="vt")
                nc.tensor.matmul(
                    out=vtp[:], lhsT=AvD[:], rhs=hT[:, 512 * q2 : 512 * (q2 + 1)]
                )
                vs_top = vs[0:64, :].rearrange("p (q r) -> p q r", r=66)
                vs_bot = vs[64:128, :].rearrange("p (q r) -> p q r", r=66)
                nc.scalar.activation(
                    out=vs_top[:, 16 * q2 : 16 * (q2 + 1), 0:32],
                    in_=vtp[0:64, :].rearrange("p (q n) -> p q n", n=32),
                    func=AF.Silu,
                )
                nc.scalar.activation(
                    out=vs_bot[:, 16 * q2 : 16 * (q2 + 1), 33:65],
                    in_=vtp[64:128, :].rearrange("p (q n) -> p q n", n=32),
                    func=AF.Silu,
                )

                # ---- spread K into block-sparse rhs (8 group slots of 256)
                ks = ksp[q2 % 2]
                for c in range(4):
                    src = kts[32 * c : 32 * c + 32, :].rearrange(
                        "p (g j) -> p g j", j=128
                    )
                    dst = ks[32 * c : 32 * c + 32, :].rearrange(
                        "p (g r) -> p g r", r=256
                    )
                    nc.vector.tensor_copy(
                        out=dst[:, 0:4, 64 * c : 64 * c + 64], in_=src[:, :, 0:64]
                    )
                    nc.vector.tensor_copy(
                        out=dst[:, 4:8, 64 * c : 64 * c + 64], in_=src[:, :, 64:128]
                    )

                # ---- S1 matmuls: (i, j)-orientation, odd groups column-shifted
                s1t = s1pool.tile([128, 1024], F32, tag="s1")
                nc.tensor.matmul(out=s1t[0:64, 0:1], lhsT=ks[:, 0:64], rhs=ks[:, 0:1])
                for g in range(4):
                    nc.tensor.matmul(
                        out=s1t[0:64, 256 * g : 256 * (g + 1)],
                        lhsT=qts[:, 128 * g : 128 * g + 64],
                        rhs=ks[:, 256 * g : 256 * (g + 1)],
                    )
                    gs = (g + 1) % 4
                    nc.tensor.matmul(
                        out=s1t[64:128, 256 * gs : 256 * (gs + 1)],
                        lhsT=qts[:, 128 * g + 64 : 128 * (g + 1)],
                        rhs=ks[:, 256 * (4 + g) : 256 * (5 + g)],
                    )

                # ---- softmax: rowmax (negated), subtract, exp -> P1c (bf16)
                negm = spool.tile([128, 16], F32, tag="negm")
                s1v = s1t[:].rearrange("p (b c j) -> p b c j", c=4, j=64)
                nc.vector.tensor_reduce(
                    out=negm[:], in_=s1v, axis=AX.X, op=ALU.max, negate=True
                )
                nmv = (
                    negm[:]
                    .rearrange("p (b c) -> p b c", c=4)
                    .unsqueeze(3)
                    .broadcast_to([128, 4, 4, 64])
                )
                nc.vector.tensor_tensor(out=s1v, in0=s1v, in1=nmv, op=ALU.add)
                P1c = spool.tile([128, 1024], BF16, tag="p1c")
                nc.scalar.activation(out=P1c[:], in_=s1t[:], func=AF.Exp)

                # rearrange to pair-adjacent layout with junk cols:
                # pair p = 4*d + c at cols [128p, 128p+128):
                #   [0:64] top = even batch (ge_d, c) ; [64:128] bottom = odd
                #   batch (go_d, c) which exp wrote at col-block (d+1)%4.
                P1q = p1q[q2 % 2]
                c_top = P1c[0:64, :].rearrange("p (d c j) -> p d c j", c=4, j=64)
                q_top = P1q[0:64, :].rearrange("p (d c j) -> p d c j", c=4, j=128)
                nc.vector.tensor_copy(out=q_top[:, :, :, 0:64], in_=c_top)
                c_ba = P1c[64:128, 256:1024].rearrange(
                    "p (d c j) -> p d c j", c=4, j=64
                )
                q_bot = P1q[64:128, :].rearrange("p (d c j) -> p d c j", c=4, j=128)
                nc.vector.tensor_copy(out=q_bot[:, 0:3, :, 64:128], in_=c_ba)
                c_bb = P1c[64:128, 0:256].rearrange("p (c j) -> p c j", j=64)
                nc.vector.tensor_copy(out=q_bot[:, 3, :, 64:128], in_=c_bb)

                # ---- P transposes (junk-tolerant) + attn matmuls + norm
                for dd in range(4):
                    ptp = ppool.tile([128, 512], BF16, tag="ptr")
                    atp = apool.tile([128, 264], F32, tag="at")
                    if dd == 0:
                        nc.tensor.matmul(
                            out=atp[0:128, 0:1], lhsT=vs[:, 0:128], rhs=vs[:, 0:1]
                        )
                    for c in range(4):
                        p_loc = 4 * dd + c
                        nc.tensor.matmul(
                            out=ptp[:, 128 * c : 128 * (c + 1)],
                            lhsT=P1q[:, 128 * p_loc : 128 * (p_loc + 1)],
                            rhs=id128[:],
                            is_transpose=True,
                        )
                    p2s = spool.tile([128, 512], BF16, tag="p2s")
                    nc.vector.tensor_copy(out=p2s[:], in_=ptp[:])
                    for c in range(4):
                        P = 16 * q2 + 4 * dd + c  # pair index within supertile
                        nc.tensor.matmul(
                            out=atp[:, 66 * c : 66 * (c + 1)],
                            lhsT=p2s[:, 128 * c : 128 * (c + 1)],
                            rhs=vs[:, 66 * P : 66 * (P + 1)],
                        )
                    # normalize 4 pairs: recip(Z), scale valid halves into anrm
                    zr = spool.tile([128, 8], F32, tag="zr")
                    av4 = atp[:].rearrange("p (q s r) -> p q s r", s=2, r=33)
                    zrv = zr[:].rearrange("p (q s) -> p q s", s=2).unsqueeze(3)
                    nc.vector.reciprocal(out=zrv, in_=av4[:, :, :, 32:33])
                    p0 = 16 * q2 + 4 * dd
                    an_top = anrm[0:64, :].rearrange("p (P n) -> p P n", n=32)
                    an_bot = anrm[64:128, :].rearrange("p (P n) -> p P n", n=32)
                    nc.vector.tensor_tensor(
                        out=an_top[:, p0 : p0 + 4, :],
                        in0=av4[0:64, :, 0, 0:32],
                        in1=zrv[0:64, :, 0:1, 0].broadcast_to([64, 4, 32]),
                        op=ALU.mult,
                    )
                    nc.vector.tensor_tensor(
                        out=an_bot[:, p0 : p0 + 4, :],
                        in0=av4[64:128, :, 1, 0:32],
                        in1=zrv[64:128, :, 1:2, 0].broadcast_to([64, 4, 32]),
                        op=ALU.mult,
                    )

            # ---- attn silu, y projection, lap*mean
            nc.scalar.activation(out=asil[:, 0:1024], in_=anrm[:, 0:1024], func=AF.Silu)
            nc.scalar.activation(
                out=asil[:, 1024:2048], in_=anrm[:, 1024:2048], func=AF.Silu
            )

            ysil = spool.tile([16, 2048], F32, tag="ysil")
            for k in range(4):
                ytp = ppool.tile([128, 512], F32, tag="hty")
                nc.tensor.matmul(
                    out=ytp[0:16, :],
                    lhsT=WoutD[:],
                    rhs=asil[:, 512 * k : 512 * (k + 1)],
                )
                nc.scalar.activation(
                    out=ysil[:, 512 * k : 512 * (k + 1)],
                    in_=ytp[0:16, :],
                    func=AF.Silu,
                    bias=bout2[:],
                )
            lap16 = spool.tile([16, 2048], F32, tag="lap16")
            lsrc = bass.AP(lap[:].tensor, tok0, [[THALF, 2], [0, 8], [1, 2048]])
            nc.gpsimd.dma_start(
                out=lap16[:].rearrange("p (a t) -> p a t", a=1), in_=lsrc
            )
            nc.vector.tensor_scalar_mul(lap16[:], lap16[:], 1.0 / 32.0)
            nc.vector.tensor_tensor(out=ysil[:], in0=ysil[:], in1=lap16[:], op=ALU.mult)
            nc.vector.tensor_reduce(
                out=yall[:, 64 * s : 64 * (s + 1)],
                in_=ysil[:].rearrange("p (b n) -> p b n", n=32),
                axis=AX.X,
                op=ALU.add,
            )

        # ---------------- epilogue: write yall directly ----------------
        nc.sync.dma_start(out=out[:], in_=yall[:])

    nc.compile()
    return nc


# ---------------------------------------------------------------------------
# host-side packing


def make_in_map(xc, lapc, W_in, b_in, Aq, Ak, Av, W_out, b_out):
    """Per-core input dict. xc: (bs, 32, 8); lapc: (bs, 32)."""
    import ml_dtypes

    bs = xc.shape[0]
    half = bs // 2
    xf = np.ascontiguousarray(xc, dtype=np.float32).reshape(2, half * 32, 8)
    xT = np.concatenate([xf[0].T, xf[1].T], axis=0)  # (16, half*32)

    def diag2(m):
        d = np.zeros((2 * m.shape[0], 2 * m.shape[1]), m.dtype)
        d[: m.shape[0], : m.shape[1]] = m
        d[m.shape[0] :, m.shape[1] :] = m
        return d

    return {
        "xT": np.ascontiguousarray(xT, dtype=np.float32),
        "lap": np.ascontiguousarray(
            lapc.reshape(2, half * 32), dtype=np.float32
        ),
        "W2d": diag2(np.asarray(W_in, np.float32).T),
        "bin2d": np.concatenate([b_in, b_in]).reshape(128, 1).astype(np.float32),
        "AqDd": diag2(np.asarray(Aq, np.float32).T),
        "AkDd": diag2(np.asarray(Ak, np.float32).T),
        "AvDd": diag2(np.asarray(Av, np.float32).T),
        "WoDd": diag2(np.asarray(W_out, np.float32).T).astype(ml_dtypes.bfloat16),
        "bout2d": np.concatenate([b_out, b_out]).reshape(16, 1).astype(np.float32),
    }


def unpack_out(raw):
    """raw: (16, half) -> (bs, 8): out[h*half + q, o] = raw[8h + o, q]."""
    half = raw.shape[1]
    return raw.reshape(2, 8, half).transpose(0, 2, 1).reshape(2 * half, 8)


# ---------------------------------------------------------------------------
# cached SPMD runner (replicates bass2jax.run_bass_via_pjrt with jit caching)

_CACHE: dict = {}


def _get_runner(reps: int = 1):
    key = ("runner", reps)
    if key in _CACHE:
        return _CACHE[key]
    import jax
    from jax.sharding import Mesh, PartitionSpec
    from jax.experimental.shard_map import shard_map
    from concourse import bass2jax

    bass2jax.install_neuronx_cc_hook()
    nc = build_nc(BS, reps=reps)

    pname = nc.partition_id_tensor.name if nc.partition_id_tensor else None
    in_names: list[str] = []
    out_names: list[str] = []
    out_avals = []
    for alloc in nc.m.functions[0].allocations:
        if not isinstance(alloc, mybir.MemoryLocationSet):
            continue
        name = alloc.memorylocations[0].name
        if alloc.kind == "ExternalInput":
            if name != pname:
                in_names.append(name)
        elif alloc.kind == "ExternalOutput":
            out_names.append(name)
            out_avals.append(
                jax.core.ShapedArray(
                    tuple(alloc.tensor_shape), mybir.dt.np(alloc.dtype)
                )
            )
    n_params = len(in_names)
    all_names = in_names + out_names
    if pname is not None:
        all_names = all_names + [pname]

    def _body(*args):
        operands = list(args)
        if pname is not None:
            operands.append(bass2jax.partition_id_tensor())
        outs = bass2jax._bass_exec_p.bind(
            *operands,
            out_avals=tuple(out_avals),
            in_names=tuple(all_names),
            out_names=tuple(out_names),
            lowering_input_output_aliases=(),
            sim_require_finite=True,
            sim_require_nnan=True,
            nc=nc,
        )
        return tuple(outs)

    devices = jax.devices()[:NCORES]
    mesh = Mesh(np.asarray(devices), ("core",))
    n_outs = len(out_names)
    sharded = jax.jit(
        shard_map(
            _body,
            mesh=mesh,
            in_specs=(PartitionSpec("core"),) * (n_params + n_outs),
            out_specs=(PartitionSpec("core"),) * n_outs,
            check_rep=False,
        ),
        donate_argnums=tuple(range(n_params, n_params + n_outs)),
        keep_unused=True,
    )
    out_shapes = [tuple(a.shape) for a in out_avals]
    out_dtypes = [a.dtype for a in out_avals]
    runner = (sharded, in_names, out_names, out_shapes, out_dtypes)
    _CACHE[key] = runner
    return runner


def run_spmd(in_maps, reps: int = 1):
    sharded, in_names, out_names, out_shapes, out_dtypes = _get_runner(reps)
    concat_in = [
        np.concatenate([in_maps[c][n] for c in range(NCORES)], axis=0)
        for n in in_names
    ]
    concat_zero = [
        np.zeros((NCORES * s[0],) + s[1:], d) for s, d in zip(out_shapes, out_dtypes)
    ]
    outs = sharded(*concat_in, *concat_zero)
    o = np.asarray(outs[0])
    per_core = o.reshape(NCORES, *out_shapes[0])
    return per_core


def kernel(x, laplacian, W_in, b_in, Aq, Ak, Av, W_out, b_out):
    x = np.asarray(x, dtype=np.float32).reshape(NCORES, BS, NA, DIN)
    lap = np.asarray(laplacian, dtype=np.float32).reshape(NCORES, BS, NA)
    args = tuple(
        np.asarray(a)
        for a in (W_in, b_in, Aq, Ak, Av, W_out, b_out)
    )
    in_maps = [make_in_map(x[c], lap[c], *args) for c in range(NCORES)]
    raws = run_spmd(in_maps)
    out = np.concatenate([unpack_out(raws[c]) for c in range(NCORES)], axis=0)
    return out.reshape(-1, NA, DOUT).astype(np.float32)


if __name__ == "__main__":
    import reference

    ins = {k: np.asarray(v) for k, v in reference.setup_inputs().items()}
    exp = np.asarray(reference.reference(**ins))
    got = kernel(**ins)
    err = np.abs(got - exp).max() / (np.abs(exp).max() + 1e-9)
    print("shapes", got.shape, exp.shape, "relerr", err)


# revision 10
# speedup vs baseline: 5.0635x; 5.0635x over previous
"""Trainium2 Bass kernel for nn_Attention_preprocessor (gnn_message_passing).

Pure data parallel: batch dim B=8192 sharded across the 8 NeuronCores
(1024 batches/core); the small weight matrices are replicated.  The
per-core program is a hand-written Bass/Tile kernel (see bass_kernel.py
when present; the builder is inlined below so this file is self-contained):

  h  = silu(x @ W_in.T + b_in) * lap
  Q/K (token layout, f32 - they feed exp), V (hidden layout, bf16)
  S[i,j] = sum_n Q[n,i] K[n,j]; P = exp(S - rowmax); attn = (P @ V)/Z
  y = silu(attn @ W_out.T + b_out) * lap; out = mean_n y

Layout highlights: batches pair (b, b+B/2) on SBUF partition halves
("2-stack"), diagonal-duplicated weights make every shared matmul dense,
S is computed in (i,j) orientation for free-axis softmax reductions, P is
transposed per pair on the TensorEngine with junk-tolerant (128,128)
transposes, and the attention matmul consumes a zero-structured V-spread
with ones-columns that produce the softmax denominators for free.
"""

from contextlib import ExitStack

import numpy as np

import concourse.bass as bass
import concourse.bacc as bacc
import concourse.mybir as mybir
from concourse.tile import TileContext

F32 = mybir.dt.float32
BF16 = mybir.dt.bfloat16
I32 = mybir.dt.int32
AF = mybir.ActivationFunctionType
ALU = mybir.AluOpType
AX = mybir.AxisListType

NCORES = 8
B, NA, DIN, H, DOUT = 8192, 32, 8, 64, 8
BS = B // NCORES


def build_nc(bs: int, reps: int = 1, stages: int = 5, loop_reps: int = 1) -> bass.Bass:
    assert bs % 256 == 0
    HALF = bs // 2
    NST = bs // 128  # supertiles of 128 batches (64 even-half + 64 odd-half)
    THALF = HALF * 32  # tokens per half

    nc = bacc.Bacc("TRN2", target_bir_lowering=False, debug=False)

    # xT: [16, THALF] f32: rows 0:8 = x^T of even-half tokens, 8:16 = odd-half
    x = nc.declare_dram_parameter("xT", [16, THALF], F32, isOutput=False)
    lap = nc.declare_dram_parameter("lap", [2, THALF], F32, isOutput=False)
    w2d = nc.declare_dram_parameter("W2d", [16, 128], F32, isOutput=False)
    bin2d = nc.declare_dram_parameter("bin2d", [128, 1], F32, isOutput=False)
    aqd = nc.declare_dram_parameter("AqDd", [128, 128], F32, isOutput=False)
    akd = nc.declare_dram_parameter("AkDd", [128, 128], F32, isOutput=False)
    avd = nc.declare_dram_parameter("AvDd", [128, 128], F32, isOutput=False)
    wod = nc.declare_dram_parameter("WoDd", [128, 16], BF16, isOutput=False)
    bout2d = nc.declare_dram_parameter("bout2d", [16, 1], F32, isOutput=False)
    # out rows = (half, o), cols = batch within half; host reassembles
    out = nc.declare_dram_parameter("out", [16, HALF], F32, isOutput=True)

    with ExitStack() as ctx:
        tc = ctx.enter_context(TileContext(nc))
        cpool = ctx.enter_context(tc.tile_pool(name="const", bufs=1))
        spool = ctx.enter_context(tc.tile_pool(name="sb", bufs=2))
        ppool = ctx.enter_context(tc.tile_pool(name="ps", bufs=1, space="PSUM"))
        s1pool = ctx.enter_context(tc.tile_pool(name="s1p", bufs=1, space="PSUM"))
        apool = ctx.enter_context(tc.tile_pool(name="ap", bufs=1, space="PSUM"))

        # ---------------- constants (host-prebuilt, one DMA each) --------
        W2 = cpool.tile([16, 128], F32, tag="w2")
        nc.sync.dma_start(out=W2[:], in_=w2d[:])
        AqD = cpool.tile([128, 128], F32, tag="aqd")
        nc.sync.dma_start(out=AqD[:], in_=aqd[:])
        AkD = cpool.tile([128, 128], F32, tag="akd")
        nc.sync.dma_start(out=AkD[:], in_=akd[:])
        AvD = cpool.tile([128, 128], F32, tag="avd")
        nc.sync.dma_start(out=AvD[:], in_=avd[:])
        WoutD = cpool.tile([128, 16], BF16, tag="woutd")
        nc.sync.dma_start(out=WoutD[:], in_=wod[:])
        bin2 = cpool.tile([128, 1], F32, tag="bin2")
        nc.sync.dma_start(out=bin2[:], in_=bin2d[:])
        bout2 = cpool.tile([16, 1], F32, tag="bout2")
        nc.sync.dma_start(out=bout2[:], in_=bout2d[:])

        # identities for PE transposes
        itf = cpool.tile([128, 128], I32, tag="itf")
        itp = cpool.tile([128, 128], I32, tag="itp")
        nc.gpsimd.iota(itf[:], [[1, 128]], channel_multiplier=0)
        nc.gpsimd.iota(itp[:], [[0, 128]], channel_multiplier=1)
        id128 = cpool.tile([128, 128], BF16, tag="id128")
        nc.vector.tensor_tensor(out=id128[:], in0=itf[:], in1=itp[:], op=ALU.is_equal)

        # PE semaphore warm-up: matmuls lower to LDW+MM with tight wait
        # budgets; pre-observe each constant's DMA semaphore.
        dmy = ppool.tile([128, 512], F32, tag="hty")
        nc.tensor.matmul(out=dmy[0:128, 0:1], lhsT=W2[:], rhs=W2[:, 0:1])
        nc.tensor.matmul(out=dmy[0:128, 1:2], lhsT=AqD[:], rhs=AqD[:, 0:1])
        nc.tensor.matmul(out=dmy[0:128, 2:3], lhsT=AkD[:], rhs=AkD[:, 0:1])
        nc.tensor.matmul(out=dmy[0:128, 3:4], lhsT=AvD[:], rhs=AvD[:, 0:1])
        nc.tensor.matmul(out=dmy[0:16, 4:5], lhsT=WoutD[:], rhs=WoutD[:, 0:1])
        nc.tensor.matmul(out=dmy[0:128, 5:6], lhsT=id128[:], rhs=id128[:, 0:1])

        # K-spread ping-pong: zero background persists, K blocks overwritten
        ksp = []
        for i in range(2):
            t = cpool.tile([128, 2048], F32, tag=f"ksp{i}")
            nc.vector.memset(t[:], 0.0)
            ksp.append(t)

        # pair-adjacent P (post-exp) ping-pong; junk quadrants zeroed once
        p1q = []
        for i in range(2):
            t = cpool.tile([128, 2048], BF16, tag=f"p1q{i}")
            nc.vector.memset(t[:], 0.0)
            p1q.append(t)

        # V-spread ping-pong: per-pair slot of 66 cols:
        #   [0:32]=V_be (top), [32]=ones(top), [33:65]=V_bo (bottom), [65]=ones(bottom)
        vsp = []
        for i in range(2):
            t = cpool.tile([128, 66 * 64], BF16, tag=f"vsp{i}")
            nc.vector.memset(t[:], 0.0)
            tv_top = t[0:64, :].rearrange("p (q r) -> p q r", r=66)
            tv_bot = t[64:128, :].rearrange("p (q r) -> p q r", r=66)
            nc.vector.memset(tv_top[:, :, 32:33], 1.0)
            nc.vector.memset(tv_bot[:, :, 65:66], 1.0)
            vsp.append(t)

        yall = cpool.tile([16, bs // 2], F32, tag="yall")
        nc.vector.memset(yall[:], 0.0)

        # ---------------- main loop over supertiles ----------------
        if loop_reps > 1:
            ctx.enter_context(tc.For_i(0, loop_reps, 1))
        for s_rep in range(NST * reps):
            s = s_rep % NST
            tok0 = 2048 * s  # token offset within each half

            xT2 = spool.tile([16, 2048], F32, tag="xt2")
            nc.gpsimd.dma_start(out=xT2[:], in_=x[:, tok0 : tok0 + 2048])

            lap128 = spool.tile([128, 2048], F32, tag="lap128")
            lsrc = bass.AP(lap[:].tensor, tok0, [[THALF, 2], [0, 64], [1, 2048]])
            nc.gpsimd.dma_start(
                out=lap128[:].rearrange("p (a t) -> p a t", a=1), in_=lsrc
            )

            # ---- layer 1: hT = silu(.) * lap, (128, 2048) f32, 2-stacked
            hT = spool.tile([128, 2048], F32, tag="ht")
            for q in range(4):
                htp = ppool.tile([128, 512], F32, tag="hty")
                nc.tensor.matmul(
                    out=htp[:], lhsT=W2[:], rhs=xT2[:, 512 * q : 512 * (q + 1)]
                )
                nc.scalar.activation(
                    out=hT[:, 512 * q : 512 * (q + 1)],
                    in_=htp[:],
                    func=AF.Silu,
                    bias=bin2[:],
                )
            nc.vector.tensor_tensor(out=hT[:], in0=hT[:], in1=lap128[:], op=ALU.mult)

            anrm = spool.tile([128, 2048], BF16, tag="anrm")
            asil = spool.tile([128, 2048], BF16, tag="asil")
            vs = vsp[s_rep % 2]

            for q2 in range(4 if stages >= 2 else 0):
                # ---- Qt / K token-layout chunks (4 chunks of 128 token-pairs)
                qtp = ppool.tile([128, 512], F32, tag="qt")
                ktp = ppool.tile([128, 512], F32, tag="kt")
                for cc in range(4):
                    chunk = 4 * q2 + cc
                    hchunk = hT[:, 128 * chunk : 128 * (chunk + 1)]
                    nc.tensor.matmul(
                        out=qtp[:, 128 * cc : 128 * (cc + 1)], lhsT=hchunk, rhs=AqD[:]
                    )
                    nc.tensor.matmul(
                        out=ktp[:, 128 * cc : 128 * (cc + 1)], lhsT=hchunk, rhs=AkD[:]
                    )
                qts = spool.tile([128, 512], F32, tag="qts")
                kts = spool.tile([128, 512], F32, tag="kts")
                nc.scalar.activation(out=qts[:], in_=qtp[:], func=AF.Silu)
                nc.scalar.activation(out=kts[:], in_=ktp[:], func=AF.Silu)

                # ---- V in hidden-layout (2-stacked) with fused spread-evict
                vtp = ppool.tile([128, 512], F32, tag="vt")
                nc.tensor.matmul(
                    out=vtp[:], lhsT=AvD[:], rhs=hT[:, 512 * q2 : 512 * (q2 + 1)]
                )
                vs_top = vs[0:64, :].rearrange("p (q r) -> p q r", r=66)
                vs_bot = vs[64:128, :].rearrange("p (q r) -> p q r", r=66)
                nc.scalar.activation(
                    out=vs_top[:, 16 * q2 : 16 * (q2 + 1), 0:32],
                    in_=vtp[0:64, :].rearrange("p (q n) -> p q n", n=32),
                    func=AF.Silu,
                )
                nc.scalar.activation(
                    out=vs_bot[:, 16 * q2 : 16 * (q2 + 1), 33:65],
                    in_=vtp[64:128, :].rearrange("p (q n) -> p q n", n=32),
                    func=AF.Silu,
                )

                # ---- spread K into block-sparse rhs (8 group slots of 256)
                if stages < 3:
                    continue
                ks = ksp[q2 % 2]
                for c in range(4):
                    src = kts[32 * c : 32 * c + 32, :].rearrange(
                        "p (g j) -> p g j", j=128
                    )
                    dst = ks[32 * c : 32 * c + 32, :].rearrange(
                        "p (g r) -> p g r", r=256
                    )
                    nc.gpsimd.tensor_copy(
                        out=dst[:, 0:4, 64 * c : 64 * c + 64], in_=src[:, :, 0:64]
                    )
                    nc.gpsimd.tensor_copy(
                        out=dst[:, 4:8, 64 * c : 64 * c + 64], in_=src[:, :, 64:128]
                    )

                # ---- S1 matmuls: (i, j)-orientation, odd groups column-shifted
                s1t = s1pool.tile([128, 1024], F32, tag="s1")
                nc.tensor.matmul(out=s1t[0:64, 0:1], lhsT=ks[:, 0:64], rhs=ks[:, 0:1])
                for g in range(4):
                    nc.tensor.matmul(
                        out=s1t[0:64, 256 * g : 256 * (g + 1)],
                        lhsT=qts[:, 128 * g : 128 * g + 64],
                        rhs=ks[:, 256 * g : 256 * (g + 1)],
                    )
                    gs = (g + 1) % 4
                    nc.tensor.matmul(
                        out=s1t[64:128, 256 * gs : 256 * (gs + 1)],
                        lhsT=qts[:, 128 * g + 64 : 128 * (g + 1)],
                        rhs=ks[:, 256 * (4 + g) : 256 * (5 + g)],
                    )

                # ---- softmax: rowmax (negated), subtract, exp -> P1c (bf16)
                negm = spool.tile([128, 16], F32, tag="negm")
                s1v = s1t[:].rearrange("p (b c j) -> p b c j", c=4, j=64)
                nc.vector.tensor_reduce(
                    out=negm[:], in_=s1v, axis=AX.X, op=ALU.max, negate=True
                )
                nmv = (
                    negm[:]
                    .rearrange("p (b c) -> p b c", c=4)
                    .unsqueeze(3)
                    .broadcast_to([128, 4, 4, 64])
                )
                nc.vector.tensor_tensor(out=s1v, in0=s1v, in1=nmv, op=ALU.add)
                P1c = spool.tile([128, 1024], BF16, tag="p1c")
                nc.scalar.activation(out=P1c[:], in_=s1t[:], func=AF.Exp)

                if stages < 4:
                    continue
                # rearrange to pair-adjacent layout with junk cols:
                # pair p = 4*d + c at cols [128p, 128p+128):
                #   [0:64] top = even batch (ge_d, c) ; [64:128] bottom = odd
                #   batch (go_d, c) which exp wrote at col-block (d+1)%4.
                P1q = p1q[q2 % 2]
                c_top = P1c[0:64, :].rearrange("p (d c j) -> p d c j", c=4, j=64)
                q_top = P1q[0:64, :].rearrange("p (d c j) -> p d c j", c=4, j=128)
                nc.vector.tensor_copy(out=q_top[:, :, :, 0:64], in_=c_top)
                c_ba = P1c[64:128, 256:1024].rearrange(
                    "p (d c j) -> p d c j", c=4, j=64
                )
                q_bot = P1q[64:128, :].rearrange("p (d c j) -> p d c j", c=4, j=128)
                nc.vector.tensor_copy(out=q_bot[:, 0:3, :, 64:128], in_=c_ba)
                c_bb = P1c[64:128, 0:256].rearrange("p (c j) -> p c j", j=64)
                nc.vector.tensor_copy(out=q_bot[:, 3, :, 64:128], in_=c_bb)

                # ---- P transposes (junk-tolerant) + attn matmuls + norm
                for dd in range(4):
                    ptp = ppool.tile([128, 512], BF16, tag="ptr")
                    atp = apool.tile([128, 264], F32, tag="at")
                    if dd == 0:
                        nc.tensor.matmul(
                            out=atp[0:128, 0:1], lhsT=vs[:, 0:128], rhs=vs[:, 0:1]
                        )
                    for c in range(4):
                        p_loc = 4 * dd + c
                        nc.tensor.matmul(
                            out=ptp[:, 128 * c : 128 * (c + 1)],
                            lhsT=P1q[:, 128 * p_loc : 128 * (p_loc + 1)],
                            rhs=id128[:],
                            is_transpose=True,
                        )
                    p2s = spool.tile([128, 512], BF16, tag="p2s")
                    if dd % 2 == 0:
                        nc.vector.tensor_copy(out=p2s[:], in_=ptp[:])
                    else:
                        nc.scalar.copy(out=p2s[:], in_=ptp[:])
                    for c in range(4):
                        P = 16 * q2 + 4 * dd + c  # pair index within supertile
                        nc.tensor.matmul(
                            out=atp[:, 66 * c : 66 * (c + 1)],
                            lhsT=p2s[:, 128 * c : 128 * (c + 1)],
                            rhs=vs[:, 66 * P : 66 * (P + 1)],
                        )
                    # normalize 4 pairs: recip(Z), scale valid halves into anrm
                    zr = spool.tile([128, 8], F32, tag="zr")
                    av4 = atp[:].rearrange("p (q s r) -> p q s r", s=2, r=33)
                    zrv = zr[:].rearrange("p (q s) -> p q s", s=2).unsqueeze(3)
                    nc.vector.reciprocal(out=zrv, in_=av4[:, :, :, 32:33])
                    p0 = 16 * q2 + 4 * dd
                    an_top = anrm[0:64, :].rearrange("p (P n) -> p P n", n=32)
                    an_bot = anrm[64:128, :].rearrange("p (P n) -> p P n", n=32)
                    nc.vector.tensor_tensor(
                        out=an_top[:, p0 : p0 + 4, :],
                        in0=av4[0:64, :, 0, 0:32],
                        in1=zrv[0:64, :, 0:1, 0].broadcast_to([64, 4, 32]),
                        op=ALU.mult,
                    )
                    nc.vector.tensor_tensor(
                        out=an_bot[:, p0 : p0 + 4, :],
                        in0=av4[64:128, :, 1, 0:32],
                        in1=zrv[64:128, :, 1:2, 0].broadcast_to([64, 4, 32]),
                        op=ALU.mult,
                    )

            # ---- attn silu, y projection, lap*mean
            if stages < 5:
                continue
            nc.scalar.activation(out=asil[:, 0:1024], in_=anrm[:, 0:1024], func=AF.Silu)
            nc.scalar.activation(
                out=asil[:, 1024:2048], in_=anrm[:, 1024:2048], func=AF.Silu
            )

            ysil = spool.tile([16, 2048], F32, tag="ysil")
            for k in range(4):
                ytp = ppool.tile([128, 512], F32, tag="hty")
                nc.tensor.matmul(
                    out=ytp[0:16, :],
                    lhsT=WoutD[:],
                    rhs=asil[:, 512 * k : 512 * (k + 1)],
                )
                nc.scalar.activation(
                    out=ysil[:, 512 * k : 512 * (k + 1)],
                    in_=ytp[0:16, :],
                    func=AF.Silu,
                    bias=bout2[:],
                )
            lap16 = spool.tile([16, 2048], F32, tag="lap16")
            lsrc = bass.AP(lap[:].tensor, tok0, [[THALF, 2], [0, 8], [1, 2048]])
            nc.gpsimd.dma_start(
                out=lap16[:].rearrange("p (a t) -> p a t", a=1), in_=lsrc
            )
            nc.vector.tensor_scalar_mul(lap16[:], lap16[:], 1.0 / 32.0)
            nc.vector.tensor_tensor(out=ysil[:], in0=ysil[:], in1=lap16[:], op=ALU.mult)
            nc.vector.tensor_reduce(
                out=yall[:, 64 * s : 64 * (s + 1)],
                in_=ysil[:].rearrange("p (b n) -> p b n", n=32),
                axis=AX.X,
                op=ALU.add,
            )

        # ---------------- epilogue: write yall directly ----------------
        nc.sync.dma_start(out=out[:], in_=yall[:])

    nc.compile()
    return nc


# ---------------------------------------------------------------------------
# host-side packing


def make_in_map(xc, lapc, W_in, b_in, Aq, Ak, Av, W_out, b_out):
    """Per-core input dict. xc: (bs, 32, 8); lapc: (bs, 32)."""
    import ml_dtypes

    bs = xc.shape[0]
    half = bs // 2
    xf = np.ascontiguousarray(xc, dtype=np.float32).reshape(2, half * 32, 8)
    xT = np.concatenate([xf[0].T, xf[1].T], axis=0)  # (16, half*32)

    def diag2(m):
        d = np.zeros((2 * m.shape[0], 2 * m.shape[1]), m.dtype)
        d[: m.shape[0], : m.shape[1]] = m
        d[m.shape[0] :, m.shape[1] :] = m
        return d

    return {
        "xT": np.ascontiguousarray(xT, dtype=np.float32),
        "lap": np.ascontiguousarray(
            lapc.reshape(2, half * 32), dtype=np.float32
        ),
        "W2d": diag2(np.asarray(W_in, np.float32).T),
        "bin2d": np.concatenate([b_in, b_in]).reshape(128, 1).astype(np.float32),
        "AqDd": diag2(np.asarray(Aq, np.float32).T),
        "AkDd": diag2(np.asarray(Ak, np.float32).T),
        "AvDd": diag2(np.asarray(Av, np.float32).T),
        "WoDd": diag2(np.asarray(W_out, np.float32).T).astype(ml_dtypes.bfloat16),
        "bout2d": np.concatenate([b_out, b_out]).reshape(16, 1).astype(np.float32),
    }


def unpack_out(raw):
    """raw: (16, half) -> (bs, 8): out[h*half + q, o] = raw[8h + o, q]."""
    half = raw.shape[1]
    return raw.reshape(2, 8, half).transpose(0, 2, 1).reshape(2 * half, 8)


# ---------------------------------------------------------------------------
# cached SPMD runner (replicates bass2jax.run_bass_via_pjrt with jit caching)

_CACHE: dict = {}


def _get_runner(reps: int = 1, stages: int = 5, loop_reps: int = 1):
    key = ("runner", reps, stages, loop_reps)
    if key in _CACHE:
        return _CACHE[key]
    import jax
    from jax.sharding import Mesh, PartitionSpec
    from jax.experimental.shard_map import shard_map
    from concourse import bass2jax

    bass2jax.install_neuronx_cc_hook()
    nc = build_nc(BS, reps=reps, stages=stages, loop_reps=loop_reps)

    pname = nc.partition_id_tensor.name if nc.partition_id_tensor else None
    in_names: list[str] = []
    out_names: list[str] = []
    out_avals = []
    for alloc in nc.m.functions[0].allocations:
        if not isinstance(alloc, mybir.MemoryLocationSet):
            continue
        name = alloc.memorylocations[0].name
        if alloc.kind == "ExternalInput":
            if name != pname:
                in_names.append(name)
        elif alloc.kind == "ExternalOutput":
            out_names.append(name)
            out_avals.append(
                jax.core.ShapedArray(
                    tuple(alloc.tensor_shape), mybir.dt.np(alloc.dtype)
                )
            )
    n_params = len(in_names)
    all_names = in_names + out_names
    if pname is not None:
        all_names = all_names + [pname]

    def _body(*args):
        operands = list(args)
        if pname is not None:
            operands.append(bass2jax.partition_id_tensor())
        outs = bass2jax._bass_exec_p.bind(
            *operands,
            out_avals=tuple(out_avals),
            in_names=tuple(all_names),
            out_names=tuple(out_names),
            lowering_input_output_aliases=(),
            sim_require_finite=True,
            sim_require_nnan=True,
            nc=nc,
        )
        return tuple(outs)

    devices = jax.devices()[:NCORES]
    mesh = Mesh(np.asarray(devices), ("core",))
    n_outs = len(out_names)
    sharded = jax.jit(
        shard_map(
            _body,
            mesh=mesh,
            in_specs=(PartitionSpec("core"),) * (n_params + n_outs),
            out_specs=(PartitionSpec("core"),) * n_outs,
            check_rep=False,
        ),
        donate_argnums=tuple(range(n_params, n_params + n_outs)),
        keep_unused=True,
    )
    out_shapes = [tuple(a.shape) for a in out_avals]
    out_dtypes = [a.dtype for a in out_avals]
    runner = (sharded, in_names, out_names, out_shapes, out_dtypes)
    _CACHE[key] = runner
    return runner


def run_spmd(in_maps, reps: int = 1, stages: int = 5, loop_reps: int = 1):
    sharded, in_names, out_names, out_shapes, out_dtypes = _get_runner(reps, stages, loop_reps)
    concat_in = [
        np.concatenate([in_maps[c][n] for c in range(NCORES)], axis=0)
        for n in in_names
    ]
    concat_zero = [
        np.zeros((NCORES * s[0],) + s[1:], d) for s, d in zip(out_shapes, out_dtypes)
    ]
    outs = sharded(*concat_in, *concat_zero)
    o = np.asarray(outs[0])
    per_core = o.reshape(NCORES, *out_shapes[0])
    return per_core


def kernel(x, laplacian, W_in, b_in, Aq, Ak, Av, W_out, b_out):
    x = np.asarray(x, dtype=np.float32).reshape(NCORES, BS, NA, DIN)
    lap = np.asarray(laplacian, dtype=np.float32).reshape(NCORES, BS, NA)
    args = tuple(
        np.asarray(a)
        for a in (W_in, b_in, Aq, Ak, Av, W_out, b_out)
    )
    in_maps = [make_in_map(x[c], lap[c], *args) for c in range(NCORES)]
    raws = run_spmd(in_maps)
    out = np.concatenate([unpack_out(raws[c]) for c in range(NCORES)], axis=0)
    return out.reshape(-1, NA, DOUT).astype(np.float32)


if __name__ == "__main__":
    import reference

    ins = {k: np.asarray(v) for k, v in reference.setup_inputs().items()}
    exp = np.asarray(reference.reference(**ins))
    got = kernel(**ins)
    err = np.abs(got - exp).max() / (np.abs(exp).max() + 1e-9)
    print("shapes", got.shape, exp.shape, "relerr", err)


def golden_core(xc, lapc, W_in, b_in, Aq, Ak, Av, W_out, b_out):
    """Numpy reference for one core. xc: (bs, 32, 8), lapc: (bs, 32)."""

    def silu(v):
        return v / (1.0 + np.exp(-v))

    h = silu(xc @ W_in.T + b_in) * lapc[..., None]
    Q = silu(np.einsum("ij,bnj->bin", Aq, h))
    Kk = silu(np.einsum("ij,bnj->bni", Ak, h))
    V = silu(np.einsum("ij,bnj->bin", Av, h))
    S = np.einsum("bin,bnj->bij", Q, Kk)
    S = S - S.max(axis=2, keepdims=True)
    P = np.exp(S)
    A = P / P.sum(axis=2, keepdims=True)
    attn = silu(np.einsum("bij,bjn->bni", A, V))
    y = silu(attn @ W_out.T + b_out) * lapc[..., None]
    return y.mean(axis=1)


# revision 11
# speedup vs baseline: 6.5686x; 1.2973x over previous
"""Trainium2 Bass kernel for nn_Attention_preprocessor (gnn_message_passing).

Pure data parallel: batch dim B=8192 sharded across the 8 NeuronCores
(1024 batches/core); the small weight matrices are replicated.  The
per-core program is a hand-written Bass/Tile kernel (see bass_kernel.py
when present; the builder is inlined below so this file is self-contained):

  h  = silu(x @ W_in.T + b_in) * lap
  Q/K (token layout, f32 - they feed exp), V (hidden layout, bf16)
  S[i,j] = sum_n Q[n,i] K[n,j]; P = exp(S - rowmax); attn = (P @ V)/Z
  y = silu(attn @ W_out.T + b_out) * lap; out = mean_n y

Layout highlights: batches pair (b, b+B/2) on SBUF partition halves
("2-stack"), diagonal-duplicated weights make every shared matmul dense,
S is computed in (i,j) orientation for free-axis softmax reductions, P is
transposed per pair on the TensorEngine with junk-tolerant (128,128)
transposes, and the attention matmul consumes a zero-structured V-spread
with ones-columns that produce the softmax denominators for free.
"""

from contextlib import ExitStack

import numpy as np

import concourse.bass as bass
import concourse.bacc as bacc
import concourse.mybir as mybir
from concourse.tile import TileContext

F32 = mybir.dt.float32
BF16 = mybir.dt.bfloat16
I32 = mybir.dt.int32
AF = mybir.ActivationFunctionType
ALU = mybir.AluOpType
AX = mybir.AxisListType

NCORES = 8
B, NA, DIN, H, DOUT = 8192, 32, 8, 64, 8
BS = B // NCORES


def build_nc(bs: int, reps: int = 1, stages: int = 5, loop_reps: int = 1) -> bass.Bass:
    assert bs % 256 == 0
    HALF = bs // 2
    NST = bs // 128  # supertiles of 128 batches (64 even-half + 64 odd-half)
    THALF = HALF * 32  # tokens per half

    nc = bacc.Bacc("TRN2", target_bir_lowering=False, debug=False)

    # xT: [16, THALF] f32: rows 0:8 = x^T of even-half tokens, 8:16 = odd-half
    x = nc.declare_dram_parameter("xT", [16, THALF], F32, isOutput=False)
    lap = nc.declare_dram_parameter("lap", [2, THALF], F32, isOutput=False)
    w2d = nc.declare_dram_parameter("W2d", [16, 128], F32, isOutput=False)
    bin2d = nc.declare_dram_parameter("bin2d", [128, 1], F32, isOutput=False)
    aqd = nc.declare_dram_parameter("AqDd", [128, 128], F32, isOutput=False)
    akd = nc.declare_dram_parameter("AkDd", [128, 128], F32, isOutput=False)
    avd = nc.declare_dram_parameter("AvDd", [128, 128], F32, isOutput=False)
    wod = nc.declare_dram_parameter("WoDd", [128, 16], BF16, isOutput=False)
    bout2d = nc.declare_dram_parameter("bout2d", [16, 1], F32, isOutput=False)
    # out rows = (half, o), cols = batch within half; host reassembles
    out = nc.declare_dram_parameter("out", [16, HALF], F32, isOutput=True)

    with ExitStack() as ctx:
        tc = ctx.enter_context(TileContext(nc))
        cpool = ctx.enter_context(tc.tile_pool(name="const", bufs=1))
        spool = ctx.enter_context(tc.tile_pool(name="sb", bufs=2))
        ppool = ctx.enter_context(tc.tile_pool(name="ps", bufs=1, space="PSUM"))
        s1pool = ctx.enter_context(tc.tile_pool(name="s1p", bufs=1, space="PSUM"))
        apool = ctx.enter_context(tc.tile_pool(name="ap", bufs=1, space="PSUM"))

        # ---------------- constants (host-prebuilt, one DMA each) --------
        W2 = cpool.tile([16, 128], F32, tag="w2")
        nc.sync.dma_start(out=W2[:], in_=w2d[:])
        AqD = cpool.tile([128, 128], F32, tag="aqd")
        nc.sync.dma_start(out=AqD[:], in_=aqd[:])
        AkD = cpool.tile([128, 128], F32, tag="akd")
        nc.sync.dma_start(out=AkD[:], in_=akd[:])
        AvD = cpool.tile([128, 128], F32, tag="avd")
        nc.sync.dma_start(out=AvD[:], in_=avd[:])
        WoutD = cpool.tile([128, 16], BF16, tag="woutd")
        nc.sync.dma_start(out=WoutD[:], in_=wod[:])
        bin2 = cpool.tile([128, 1], F32, tag="bin2")
        nc.sync.dma_start(out=bin2[:], in_=bin2d[:])
        bout2 = cpool.tile([16, 1], F32, tag="bout2")
        nc.sync.dma_start(out=bout2[:], in_=bout2d[:])

        # identities for PE transposes
        itf = cpool.tile([128, 128], I32, tag="itf")
        itp = cpool.tile([128, 128], I32, tag="itp")
        nc.gpsimd.iota(itf[:], [[1, 128]], channel_multiplier=0)
        nc.gpsimd.iota(itp[:], [[0, 128]], channel_multiplier=1)
        id128 = cpool.tile([128, 128], BF16, tag="id128")
        nc.vector.tensor_tensor(out=id128[:], in0=itf[:], in1=itp[:], op=ALU.is_equal)

        # PE semaphore warm-up: matmuls lower to LDW+MM with tight wait
        # budgets; pre-observe each constant's DMA semaphore.
        dmy = ppool.tile([128, 512], F32, tag="hty")
        nc.tensor.matmul(out=dmy[0:128, 0:1], lhsT=W2[:], rhs=W2[:, 0:1])
        nc.tensor.matmul(out=dmy[0:128, 1:2], lhsT=AqD[:], rhs=AqD[:, 0:1])
        nc.tensor.matmul(out=dmy[0:128, 2:3], lhsT=AkD[:], rhs=AkD[:, 0:1])
        nc.tensor.matmul(out=dmy[0:128, 3:4], lhsT=AvD[:], rhs=AvD[:, 0:1])
        nc.tensor.matmul(out=dmy[0:16, 4:5], lhsT=WoutD[:], rhs=WoutD[:, 0:1])
        nc.tensor.matmul(out=dmy[0:128, 5:6], lhsT=id128[:], rhs=id128[:, 0:1])

        # K-spread ping-pong: zero background persists, K blocks overwritten
        ksp = []
        for i in range(2):
            t = cpool.tile([128, 2048], F32, tag=f"ksp{i}")
            nc.vector.memset(t[:], 0.0)
            ksp.append(t)

        # pair-adjacent P (post-exp) ping-pong; junk quadrants zeroed once
        p1q = []
        for i in range(2):
            t = cpool.tile([128, 2048], BF16, tag=f"p1q{i}")
            nc.vector.memset(t[:], 0.0)
            p1q.append(t)

        # V-spread ping-pong: per-pair slot of 66 cols:
        #   [0:32]=V_be (top), [32]=ones(top), [33:65]=V_bo (bottom), [65]=ones(bottom)
        vsp = []
        for i in range(2):
            t = cpool.tile([128, 66 * 64], BF16, tag=f"vsp{i}")
            nc.vector.memset(t[:], 0.0)
            tv_top = t[0:64, :].rearrange("p (q r) -> p q r", r=66)
            tv_bot = t[64:128, :].rearrange("p (q r) -> p q r", r=66)
            nc.vector.memset(tv_top[:, :, 32:33], 1.0)
            nc.vector.memset(tv_bot[:, :, 65:66], 1.0)
            vsp.append(t)

        yall = cpool.tile([16, bs // 2], F32, tag="yall")
        nc.vector.memset(yall[:], 0.0)

        # ---------------- main loop over supertiles ----------------
        if loop_reps > 1:
            ctx.enter_context(tc.For_i(0, loop_reps, 1))
        for s_rep in range(NST * reps):
            s = s_rep % NST
            tok0 = 2048 * s  # token offset within each half

            xT2 = spool.tile([16, 2048], F32, tag="xt2")
            nc.gpsimd.dma_start(out=xT2[:], in_=x[:, tok0 : tok0 + 2048])

            lap128 = spool.tile([128, 2048], F32, tag="lap128")
            lsrc = bass.AP(lap[:].tensor, tok0, [[THALF, 2], [0, 64], [1, 2048]])
            nc.gpsimd.dma_start(
                out=lap128[:].rearrange("p (a t) -> p a t", a=1), in_=lsrc
            )

            # ---- layer 1: hT = silu(.) * lap, (128, 2048) f32, 2-stacked
            hT = spool.tile([128, 2048], F32, tag="ht")
            for q in range(4):
                htp = ppool.tile([128, 512], F32, tag="hty")
                nc.tensor.matmul(
                    out=htp[:], lhsT=W2[:], rhs=xT2[:, 512 * q : 512 * (q + 1)]
                )
                nc.scalar.activation(
                    out=hT[:, 512 * q : 512 * (q + 1)],
                    in_=htp[:],
                    func=AF.Silu,
                    bias=bin2[:],
                )
            nc.vector.tensor_tensor(out=hT[:], in0=hT[:], in1=lap128[:], op=ALU.mult)

            anrm = spool.tile([128, 2048], BF16, tag="anrm")
            asil = spool.tile([128, 2048], BF16, tag="asil")
            vs = vsp[s_rep % 2]

            for q2 in range(4 if stages >= 2 else 0):
                # ---- Qt / K token-layout chunks (4 chunks of 128 token-pairs)
                qtp = ppool.tile([128, 512], F32, tag="qt")
                ktp = ppool.tile([128, 512], F32, tag="kt")
                for cc in range(4):
                    chunk = 4 * q2 + cc
                    hchunk = hT[:, 128 * chunk : 128 * (chunk + 1)]
                    nc.tensor.matmul(
                        out=qtp[:, 128 * cc : 128 * (cc + 1)], lhsT=hchunk, rhs=AqD[:]
                    )
                    nc.tensor.matmul(
                        out=ktp[:, 128 * cc : 128 * (cc + 1)], lhsT=hchunk, rhs=AkD[:]
                    )
                qts = spool.tile([128, 512], F32, tag="qts")
                kts = spool.tile([128, 512], F32, tag="kts")
                nc.scalar.activation(out=qts[:], in_=qtp[:], func=AF.Silu)
                nc.scalar.activation(out=kts[:], in_=ktp[:], func=AF.Silu)

                # ---- V in hidden-layout (2-stacked) with fused spread-evict
                vtp = ppool.tile([128, 512], F32, tag="vt")
                nc.tensor.matmul(
                    out=vtp[:], lhsT=AvD[:], rhs=hT[:, 512 * q2 : 512 * (q2 + 1)]
                )
                vs_top = vs[0:64, :].rearrange("p (q r) -> p q r", r=66)
                vs_bot = vs[64:128, :].rearrange("p (q r) -> p q r", r=66)
                nc.scalar.activation(
                    out=vs_top[:, 16 * q2 : 16 * (q2 + 1), 0:32],
                    in_=vtp[0:64, :].rearrange("p (q n) -> p q n", n=32),
                    func=AF.Silu,
                )
                nc.scalar.activation(
                    out=vs_bot[:, 16 * q2 : 16 * (q2 + 1), 33:65],
                    in_=vtp[64:128, :].rearrange("p (q n) -> p q n", n=32),
                    func=AF.Silu,
                )

                # ---- spread K into block-sparse rhs (8 group slots of 256)
                if stages < 3:
                    continue
                ks = ksp[q2 % 2]
                for c in range(4):
                    src = kts[32 * c : 32 * c + 32, :].rearrange(
                        "p (g j) -> p g j", j=128
                    )
                    dst = ks[32 * c : 32 * c + 32, :].rearrange(
                        "p (g r) -> p g r", r=256
                    )
                    nc.vector.tensor_copy(
                        out=dst[:, 0:4, 64 * c : 64 * c + 64], in_=src[:, :, 0:64]
                    )
                    nc.vector.tensor_copy(
                        out=dst[:, 4:8, 64 * c : 64 * c + 64], in_=src[:, :, 64:128]
                    )

                # ---- S1 matmuls: (i, j)-orientation, odd groups column-shifted
                s1t = s1pool.tile([128, 1024], F32, tag="s1")
                nc.tensor.matmul(out=s1t[0:64, 0:1], lhsT=ks[:, 0:64], rhs=ks[:, 0:1])
                for g in range(4):
                    nc.tensor.matmul(
                        out=s1t[0:64, 256 * g : 256 * (g + 1)],
                        lhsT=qts[:, 128 * g : 128 * g + 64],
                        rhs=ks[:, 256 * g : 256 * (g + 1)],
                    )
                    gs = (g + 1) % 4
                    nc.tensor.matmul(
                        out=s1t[64:128, 256 * gs : 256 * (gs + 1)],
                        lhsT=qts[:, 128 * g + 64 : 128 * (g + 1)],
                        rhs=ks[:, 256 * (4 + g) : 256 * (5 + g)],
                    )

                # ---- softmax: rowmax (negated), subtract, exp -> P1c (bf16)
                negm = spool.tile([128, 16], F32, tag="negm")
                s1v = s1t[:].rearrange("p (b c j) -> p b c j", c=4, j=64)
                nc.vector.tensor_reduce(
                    out=negm[:], in_=s1v, axis=AX.X, op=ALU.max, negate=True
                )
                nmv = (
                    negm[:]
                    .rearrange("p (b c) -> p b c", c=4)
                    .unsqueeze(3)
                    .broadcast_to([128, 4, 4, 64])
                )
                nc.vector.tensor_tensor(out=s1v, in0=s1v, in1=nmv, op=ALU.add)
                P1c = spool.tile([128, 1024], BF16, tag="p1c")
                nc.scalar.activation(out=P1c[:], in_=s1t[:], func=AF.Exp)

                if stages < 4:
                    continue
                # rearrange to pair-adjacent layout with junk cols:
                # pair p = 4*d + c at cols [128p, 128p+128):
                #   [0:64] top = even batch (ge_d, c) ; [64:128] bottom = odd
                #   batch (go_d, c) which exp wrote at col-block (d+1)%4.
                P1q = p1q[q2 % 2]
                c_top = P1c[0:64, :].rearrange("p (d c j) -> p d c j", c=4, j=64)
                q_top = P1q[0:64, :].rearrange("p (d c j) -> p d c j", c=4, j=128)
                nc.vector.tensor_copy(out=q_top[:, :, :, 0:64], in_=c_top)
                c_ba = P1c[64:128, 256:1024].rearrange(
                    "p (d c j) -> p d c j", c=4, j=64
                )
                q_bot = P1q[64:128, :].rearrange("p (d c j) -> p d c j", c=4, j=128)
                nc.vector.tensor_copy(out=q_bot[:, 0:3, :, 64:128], in_=c_ba)
                c_bb = P1c[64:128, 0:256].rearrange("p (c j) -> p c j", j=64)
                nc.vector.tensor_copy(out=q_bot[:, 3, :, 64:128], in_=c_bb)

                # ---- P transposes (junk-tolerant) + attn matmuls + norm
                for dd in range(4):
                    ptp = ppool.tile([128, 512], BF16, tag="ptr")
                    atp = apool.tile([128, 264], F32, tag="at")
                    if dd == 0:
                        nc.tensor.matmul(
                            out=atp[0:128, 0:1], lhsT=vs[:, 0:128], rhs=vs[:, 0:1]
                        )
                    for c in range(4):
                        p_loc = 4 * dd + c
                        nc.tensor.matmul(
                            out=ptp[:, 128 * c : 128 * (c + 1)],
                            lhsT=P1q[:, 128 * p_loc : 128 * (p_loc + 1)],
                            rhs=id128[:],
                            is_transpose=True,
                        )
                    p2s = spool.tile([128, 512], BF16, tag="p2s")
                    nc.vector.tensor_copy(out=p2s[:], in_=ptp[:])
                    for c in range(4):
                        P = 16 * q2 + 4 * dd + c  # pair index within supertile
                        nc.tensor.matmul(
                            out=atp[:, 66 * c : 66 * (c + 1)],
                            lhsT=p2s[:, 128 * c : 128 * (c + 1)],
                            rhs=vs[:, 66 * P : 66 * (P + 1)],
                        )
                    # normalize 4 pairs: recip(Z), scale valid halves into anrm
                    zr = spool.tile([128, 8], F32, tag="zr")
                    av4 = atp[:].rearrange("p (q s r) -> p q s r", s=2, r=33)
                    zrv = zr[:].rearrange("p (q s) -> p q s", s=2).unsqueeze(3)
                    nc.vector.reciprocal(out=zrv, in_=av4[:, :, :, 32:33])
                    p0 = 16 * q2 + 4 * dd
                    an_top = anrm[0:64, :].rearrange("p (P n) -> p P n", n=32)
                    an_bot = anrm[64:128, :].rearrange("p (P n) -> p P n", n=32)
                    nc.vector.tensor_tensor(
                        out=an_top[:, p0 : p0 + 4, :],
                        in0=av4[0:64, :, 0, 0:32],
                        in1=zrv[0:64, :, 0:1, 0].broadcast_to([64, 4, 32]),
                        op=ALU.mult,
                    )
                    nc.vector.tensor_tensor(
                        out=an_bot[:, p0 : p0 + 4, :],
                        in0=av4[64:128, :, 1, 0:32],
                        in1=zrv[64:128, :, 1:2, 0].broadcast_to([64, 4, 32]),
                        op=ALU.mult,
                    )

            # ---- attn silu, y projection, lap*mean
            if stages < 5:
                continue
            nc.scalar.activation(out=asil[:, 0:1024], in_=anrm[:, 0:1024], func=AF.Silu)
            nc.scalar.activation(
                out=asil[:, 1024:2048], in_=anrm[:, 1024:2048], func=AF.Silu
            )

            ysil = spool.tile([16, 2048], F32, tag="ysil")
            for k in range(4):
                ytp = ppool.tile([128, 512], F32, tag="hty")
                nc.tensor.matmul(
                    out=ytp[0:16, :],
                    lhsT=WoutD[:],
                    rhs=asil[:, 512 * k : 512 * (k + 1)],
                )
                nc.scalar.activation(
                    out=ysil[:, 512 * k : 512 * (k + 1)],
                    in_=ytp[0:16, :],
                    func=AF.Silu,
                    bias=bout2[:],
                )
            lap16 = spool.tile([16, 2048], F32, tag="lap16")
            lsrc = bass.AP(lap[:].tensor, tok0, [[THALF, 2], [0, 8], [1, 2048]])
            nc.gpsimd.dma_start(
                out=lap16[:].rearrange("p (a t) -> p a t", a=1), in_=lsrc
            )
            nc.vector.tensor_scalar_mul(lap16[:], lap16[:], 1.0 / 32.0)
            nc.vector.tensor_tensor(out=ysil[:], in0=ysil[:], in1=lap16[:], op=ALU.mult)
            nc.vector.tensor_reduce(
                out=yall[:, 64 * s : 64 * (s + 1)],
                in_=ysil[:].rearrange("p (b n) -> p b n", n=32),
                axis=AX.X,
                op=ALU.add,
            )

        # ---------------- epilogue: write yall directly ----------------
        nc.sync.dma_start(out=out[:], in_=yall[:])

    nc.compile()
    return nc


# ---------------------------------------------------------------------------
# host-side packing


def make_in_map(xc, lapc, W_in, b_in, Aq, Ak, Av, W_out, b_out):
    """Per-core input dict. xc: (bs, 32, 8); lapc: (bs, 32)."""
    import ml_dtypes

    bs = xc.shape[0]
    half = bs // 2
    xf = np.ascontiguousarray(xc, dtype=np.float32).reshape(2, half * 32, 8)
    xT = np.concatenate([xf[0].T, xf[1].T], axis=0)  # (16, half*32)

    def diag2(m):
        d = np.zeros((2 * m.shape[0], 2 * m.shape[1]), m.dtype)
        d[: m.shape[0], : m.shape[1]] = m
        d[m.shape[0] :, m.shape[1] :] = m
        return d

    return {
        "xT": np.ascontiguousarray(xT, dtype=np.float32),
        "lap": np.ascontiguousarray(
            lapc.reshape(2, half * 32), dtype=np.float32
        ),
        "W2d": diag2(np.asarray(W_in, np.float32).T),
        "bin2d": np.concatenate([b_in, b_in]).reshape(128, 1).astype(np.float32),
        "AqDd": diag2(np.asarray(Aq, np.float32).T),
        "AkDd": diag2(np.asarray(Ak, np.float32).T),
        "AvDd": diag2(np.asarray(Av, np.float32).T),
        "WoDd": diag2(np.asarray(W_out, np.float32).T).astype(ml_dtypes.bfloat16),
        "bout2d": np.concatenate([b_out, b_out]).reshape(16, 1).astype(np.float32),
    }


def unpack_out(raw):
    """raw: (16, half) -> (bs, 8): out[h*half + q, o] = raw[8h + o, q]."""
    half = raw.shape[1]
    return raw.reshape(2, 8, half).transpose(0, 2, 1).reshape(2 * half, 8)


# ---------------------------------------------------------------------------
# cached SPMD runner (replicates bass2jax.run_bass_via_pjrt with jit caching)

_CACHE: dict = {}


def _get_runner(reps: int = 1, stages: int = 5, loop_reps: int = 1):
    key = ("runner", reps, stages, loop_reps)
    if key in _CACHE:
        return _CACHE[key]
    import jax
    from jax.sharding import Mesh, PartitionSpec
    from jax.experimental.shard_map import shard_map
    from concourse import bass2jax

    bass2jax.install_neuronx_cc_hook()
    nc = build_nc(BS, reps=reps, stages=stages, loop_reps=loop_reps)

    pname = nc.partition_id_tensor.name if nc.partition_id_tensor else None
    in_names: list[str] = []
    out_names: list[str] = []
    out_avals = []
    for alloc in nc.m.functions[0].allocations:
        if not isinstance(alloc, mybir.MemoryLocationSet):
            continue
        name = alloc.memorylocations[0].name
        if alloc.kind == "ExternalInput":
            if name != pname:
                in_names.append(name)
        elif alloc.kind == "ExternalOutput":
            out_names.append(name)
            out_avals.append(
                jax.core.ShapedArray(
                    tuple(alloc.tensor_shape), mybir.dt.np(alloc.dtype)
                )
            )
    n_params = len(in_names)
    all_names = in_names + out_names
    if pname is not None:
        all_names = all_names + [pname]

    def _body(*args):
        operands = list(args)
        if pname is not None:
            operands.append(bass2jax.partition_id_tensor())
        outs = bass2jax._bass_exec_p.bind(
            *operands,
            out_avals=tuple(out_avals),
            in_names=tuple(all_names),
            out_names=tuple(out_names),
            lowering_input_output_aliases=(),
            sim_require_finite=True,
            sim_require_nnan=True,
            nc=nc,
        )
        return tuple(outs)

    devices = jax.devices()[:NCORES]
    mesh = Mesh(np.asarray(devices), ("core",))
    n_outs = len(out_names)
    sharded = jax.jit(
        shard_map(
            _body,
            mesh=mesh,
            in_specs=(PartitionSpec("core"),) * (n_params + n_outs),
            out_specs=(PartitionSpec("core"),) * n_outs,
            check_rep=False,
        ),
        donate_argnums=tuple(range(n_params, n_params + n_outs)),
        keep_unused=True,
    )
    out_shapes = [tuple(a.shape) for a in out_avals]
    out_dtypes = [a.dtype for a in out_avals]
    runner = (sharded, in_names, out_names, out_shapes, out_dtypes)
    _CACHE[key] = runner
    return runner


def run_spmd(in_maps, reps: int = 1, stages: int = 5, loop_reps: int = 1):
    sharded, in_names, out_names, out_shapes, out_dtypes = _get_runner(reps, stages, loop_reps)
    concat_in = [
        np.concatenate([in_maps[c][n] for c in range(NCORES)], axis=0)
        for n in in_names
    ]
    concat_zero = [
        np.zeros((NCORES * s[0],) + s[1:], d) for s, d in zip(out_shapes, out_dtypes)
    ]
    outs = sharded(*concat_in, *concat_zero)
    o = np.asarray(outs[0])
    per_core = o.reshape(NCORES, *out_shapes[0])
    return per_core


def kernel(x, laplacian, W_in, b_in, Aq, Ak, Av, W_out, b_out):
    x = np.asarray(x, dtype=np.float32).reshape(NCORES, BS, NA, DIN)
    lap = np.asarray(laplacian, dtype=np.float32).reshape(NCORES, BS, NA)
    args = tuple(
        np.asarray(a)
        for a in (W_in, b_in, Aq, Ak, Av, W_out, b_out)
    )
    in_maps = [make_in_map(x[c], lap[c], *args) for c in range(NCORES)]
    raws = run_spmd(in_maps)
    out = np.concatenate([unpack_out(raws[c]) for c in range(NCORES)], axis=0)
    return out.reshape(-1, NA, DOUT).astype(np.float32)


if __name__ == "__main__":
    import reference

    ins = {k: np.asarray(v) for k, v in reference.setup_inputs().items()}
    exp = np.asarray(reference.reference(**ins))
    got = kernel(**ins)
    err = np.abs(got - exp).max() / (np.abs(exp).max() + 1e-9)
    print("shapes", got.shape, exp.shape, "relerr", err)


def golden_core(xc, lapc, W_in, b_in, Aq, Ak, Av, W_out, b_out):
    """Numpy reference for one core. xc: (bs, 32, 8), lapc: (bs, 32)."""

    def silu(v):
        return v / (1.0 + np.exp(-v))

    h = silu(xc @ W_in.T + b_in) * lapc[..., None]
    Q = silu(np.einsum("ij,bnj->bin", Aq, h))
    Kk = silu(np.einsum("ij,bnj->bni", Ak, h))
    V = silu(np.einsum("ij,bnj->bin", Av, h))
    S = np.einsum("bin,bnj->bij", Q, Kk)
    S = S - S.max(axis=2, keepdims=True)
    P = np.exp(S)
    A = P / P.sum(axis=2, keepdims=True)
    attn = silu(np.einsum("bij,bjn->bni", A, V))
    y = silu(attn @ W_out.T + b_out) * lapc[..., None]
    return y.mean(axis=1)
